# revision 3
# baseline (speedup 1.0000x reference)
"""Trainium2 Bass kernel for nn_ContinuousActor (GNN message passing actor MLP).

Strategy (pure data parallel over 8 cores, batch dim sharded, feature-major):
  - Host repacks per-pair inputs: pack(i,j) = [body(10); ones(1); A_i(24);
    A_j(24)] (K=59) where A_o = [ag_o(3); g_o(3); onehot_o(3); obj_o(15)].
    All 6 pairs then share ONE stationary phi1 weight block [59, 256]
    (bias via the ones row, one-hots as data), so the pair permutation is
    pure data movement done on host.
  - Two pairs ("duo") are packed at SBUF partitions 0..58 / 64..122 and run
    as CONCURRENT matmuls via tile_position (0,0)/(64,0): phi1 takes
    ~6 x 512-cycle spans per batch tile instead of 12.
  - phi2 relu+sum-pool fused into DVE scalar_tensor_tensor ops:
      acc = (ph2 max -b2) add acc      (= relu(ph2+b2) - b2, accumulated)
    The constant -n_shift*b2 shift is folded into the rho bias host-side.
    Remaining routes use ACT relu (true values) + adds on GPSIMD.
  - 3-stage software pipeline per tile: phi1(duo d) | phi2(duo d-1) |
    rho/heads finisher of tile t-1, so PE / ACT / DVE / GPSIMD all stay busy.
"""

import numpy as np
import ml_dtypes
from contextlib import ExitStack

import concourse.bass as bass
import concourse.mybir as mybir
import concourse.tile as tile
from concourse import bacc
from concourse.bass_utils import run_bass_kernel_spmd

F32 = mybir.dt.float32
BF16 = mybir.dt.bfloat16
RELU = mybir.ActivationFunctionType.Relu
NPBF16 = ml_dtypes.bfloat16

B_FULL = 65536
N_CORES = 8
BC = B_FULL // N_CORES  # 8192 batch rows per core
BT = 512                # batch tile (matmul free dim)
KP = 59                 # packed per-pair feature rows: 10 body + 1 ones + 24 + 24
PERMS = [(0, 1), (0, 2), (1, 0), (1, 2), (2, 0), (2, 1)]
LOG_SIG_MIN, LOG_SIG_MAX = -20.0, 2.0

# --- engine routing tunables (baked into build AND bias correction) ---
# h1 = relu(ph1) evacuation engine per pair 0..5 ('act' | 'dve')
H1_ENG = ['act', 'dve', 'act', 'dve', 'act', 'dve']
# phi2 evacuation route per (mh, pair): 'stt' = DVE fused max/add chain
# (shifted by -b2, corrected in rho bias), 'act' = ACT relu (true value,
# needs a GPSIMD/DVE add except when it is the first write of that half).
PH2_ROUTE = [['stt', 'stt', 'stt', 'stt', 'stt', 'act'],
             ['act', 'act', 'act', 'act', 'act', 'act']]
# engines for the adds of non-initial 'act'-routed pairs (consumed in order)
ADD_ENG = [['gp'], ['gp', 'gp', 'gp', 'gp', 'gp']]
# rho evacuation engine per m-half
RHO_ENG = ['act', 'act']

_CACHE = {}


def _pack_256(w):
    """[256, 256] -> [128, 512] with col block (2k+m) = w[k*128:, m*128:]."""
    out = np.empty((128, 512), dtype=np.float32)
    for k in range(2):
        for m in range(2):
            out[:, (2 * k + m) * 128:(2 * k + m + 1) * 128] = \
                w[k * 128:(k + 1) * 128, m * 128:(m + 1) * 128]
    return out


def _pack_weights(phi_w1, phi_b1, phi_w2, phi_b2, rho_w1, rho_b1,
                  mean_w, mean_b, logstd_w, logstd_b):
    f = np.float32
    W1 = np.asarray(phi_w1, f)
    blk = np.concatenate([
        W1[12:22],                          # body
        np.asarray(phi_b1, f)[None, :],     # bias via ones row
        W1[0:3], W1[6:9], W1[22:25], W1[25:40],    # A_i: ag, g, onehot, feats
        W1[3:6], W1[9:12], W1[40:43], W1[43:58],   # A_j
    ], axis=0)                              # [59, 256]
    w1 = np.zeros((128, 256), dtype=f)
    w1[0:KP] = blk
    w1[64:64 + KP] = blk

    w2 = _pack_256(np.asarray(phi_w2, f))
    b2 = np.asarray(phi_b2, f)
    nb2 = np.stack([-b2[0:128], -b2[128:256]], axis=1)  # [128, 2]
    b2p = np.stack([b2[0:128], b2[128:256]], axis=1)    # [128, 2]

    wr = _pack_256(np.asarray(rho_w1, f))
    # rho bias, corrected for the shifted ('stt') phi2 routes:
    # each stt-routed (mh, pair) contributes relu(z+b2)-b2 to acc, so
    # acc = pooled - n_shift*b2_half  =>  fold  (n_shift*b2_masked) @ rho_w1.
    c = np.zeros(256, dtype=f)
    c[0:128] = sum(1 for r in PH2_ROUTE[0] if r == 'stt') * b2[0:128]
    c[128:256] = sum(1 for r in PH2_ROUTE[1] if r == 'stt') * b2[128:256]
    brv = np.asarray(rho_b1, f) + c @ np.asarray(rho_w1, f)
    brp = np.stack([brv[0:128], brv[128:256]], axis=1)  # [128, 2]

    wh_full = np.concatenate([np.asarray(mean_w, f), np.asarray(logstd_w, f)],
                             axis=1)                      # [256, 8]
    wh = np.concatenate([wh_full[0:128, :], wh_full[128:256, :]], axis=1)
    bh = np.concatenate([np.asarray(mean_b, f),
                         np.asarray(logstd_b, f)]).reshape(1, 8)
    big = np.float32(3.0e38)
    clipb = np.empty((8, 2), dtype=f)
    clipb[0:4, 0], clipb[4:8, 0] = big, LOG_SIG_MAX   # hi (min op)
    clipb[0:4, 1], clipb[4:8, 1] = -big, LOG_SIG_MIN  # lo (max op)
    ones = np.ones((1, BT), dtype=NPBF16)
    w1, w2, wr, wh, bh = (a.astype(NPBF16) for a in (w1, w2, wr, wh, bh))
    return dict(w1=w1, w2=w2, nb2=nb2, b2p=b2p, wr=wr, brp=brp, wh=wh, bh=bh,
                clipb=clipb, ones=ones)


def _pack_xt3(obs, ag, g):
    """[3, 128, B] bf16: duo d holds pair 2d at partitions 0..58 and pair
    2d+1 at partitions 64..122, each as [body;ones;A_i;A_j]."""
    B = obs.shape[0]
    xt3 = np.zeros((3, 128, B), dtype=NPBF16)
    bodyT = obs[:, 0:10].T.astype(NPBF16)
    agT = ag.T.astype(NPBF16)
    gT = g.T.astype(NPBF16)
    objT = [obs[:, 10 + 15 * o: 25 + 15 * o].T.astype(NPBF16) for o in range(3)]

    def fill_a(d, base, o):
        xt3[d, base:base + 3] = agT[3 * o:3 * o + 3]
        xt3[d, base + 3:base + 6] = gT[3 * o:3 * o + 3]
        xt3[d, base + 6 + o] = 1.0          # one-hot row
        xt3[d, base + 9:base + 24] = objT[o]

    for d in range(3):
        for half, p in ((0, 2 * d), (64, 2 * d + 1)):
            i, j = PERMS[p]
            xt3[d, half:half + 10] = bodyT
            xt3[d, half + 10] = 1.0
            fill_a(d, half + 11, i)
            fill_a(d, half + 35, j)
    return xt3


def _build_bass(bc, bt):
    nt = bc // bt
    nc = bacc.Bacc(trn_type="TRN2")

    xt3_d = nc.dram_tensor("xt3", [3, 128, bc], BF16, kind="ExternalInput")
    w1_d = nc.dram_tensor("w1", [128, 256], BF16, kind="ExternalInput")
    w2_d = nc.dram_tensor("w2", [128, 512], BF16, kind="ExternalInput")
    nb2_d = nc.dram_tensor("nb2", [128, 2], F32, kind="ExternalInput")
    b2p_d = nc.dram_tensor("b2p", [128, 2], F32, kind="ExternalInput")
    wr_d = nc.dram_tensor("wr", [128, 512], BF16, kind="ExternalInput")
    brp_d = nc.dram_tensor("brp", [128, 2], F32, kind="ExternalInput")
    wh_d = nc.dram_tensor("wh", [128, 16], BF16, kind="ExternalInput")
    bh_d = nc.dram_tensor("bh", [1, 8], BF16, kind="ExternalInput")
    clipb_d = nc.dram_tensor("clipb", [8, 2], F32, kind="ExternalInput")
    ones_d = nc.dram_tensor("ones", [1, bt], BF16, kind="ExternalInput")
    y_d = nc.dram_tensor("y", [8, bc], F32, kind="ExternalOutput")

    AMIN, AMAX, AADD = (mybir.AluOpType.min, mybir.AluOpType.max,
                        mybir.AluOpType.add)
    ABYP = mybir.AluOpType.bypass

    with ExitStack() as ctx:
        tc = ctx.enter_context(tile.TileContext(nc))
        consts = ctx.enter_context(tc.tile_pool(name="consts", bufs=1))
        sbp = ctx.enter_context(tc.tile_pool(name="sbp", bufs=3))
        psp = ctx.enter_context(tc.tile_pool(name="psp", bufs=1, space="PSUM"))

        w1sb = consts.tile([128, 256], BF16)
        nc.sync.dma_start(out=w1sb, in_=w1_d[:, :])
        w2sb = consts.tile([128, 512], BF16)
        nc.sync.dma_start(out=w2sb, in_=w2_d[:, :])
        wrsb = consts.tile([128, 512], BF16)
        nc.sync.dma_start(out=wrsb, in_=wr_d[:, :])
        whsb = consts.tile([128, 16], BF16)
        nc.sync.dma_start(out=whsb, in_=wh_d[:, :])
        nb2sb = consts.tile([128, 2], F32)
        nc.sync.dma_start(out=nb2sb, in_=nb2_d[:, :])
        b2psb = consts.tile([128, 2], F32)
        nc.sync.dma_start(out=b2psb, in_=b2p_d[:, :])
        brpsb = consts.tile([128, 2], F32)
        nc.sync.dma_start(out=brpsb, in_=brp_d[:, :])
        bhsb = consts.tile([1, 8], BF16)
        nc.sync.dma_start(out=bhsb, in_=bh_d[:, :])
        clipsb = consts.tile([8, 2], F32)
        nc.sync.dma_start(out=clipsb, in_=clipb_d[:, :])
        ones_sb = consts.tile([1, bt], BF16)
        nc.sync.dma_start(out=ones_sb, in_=ones_d[:, :])

        def eng(name):
            return {'act': nc.scalar, 'dve': nc.vector, 'gp': nc.gpsimd}[name]

        def dma_xts(t):
            s0 = t * bt
            xts = []
            for d in range(3):
                x = sbp.tile([128, bt], BF16, tag=f"xts{d}", name=f"xts{d}",
                             bufs=2)
                nc.sync.dma_start(out=x, in_=xt3_d[d, :, s0:s0 + bt])
                xts.append(x)
            return xts

        def phi1_mms(xts_d):
            """4 MMs for one duo; the two pairs run concurrently on disjoint
            row strips of the PE array."""
            phA = psp.tile([128, 2 * bt], F32, tag="ph1", name="phA", bufs=2)
            phB = psp.tile([128, 2 * bt], F32, tag="ph1", name="phB", bufs=2)
            for mh in range(2):
                nc.tensor.matmul(
                    phA[:, mh * bt:(mh + 1) * bt],
                    w1sb[0:KP, mh * 128:(mh + 1) * 128],
                    xts_d[0:KP, :],
                    start=True, stop=True, tile_position=(0, 0),
                )
                nc.tensor.matmul(
                    phB[:, mh * bt:(mh + 1) * bt],
                    w1sb[64:64 + KP, mh * 128:(mh + 1) * 128],
                    xts_d[64:64 + KP, :],
                    start=True, stop=True, tile_position=(64, 0),
                )
            return phA, phB

        def h1_evac(ph, p):
            """h1 = relu(ph1) (bias already inside via the ones row)."""
            h1 = sbp.tile([128, 2 * bt], BF16, tag="h1", name="h1", bufs=4)
            if H1_ENG[p] == 'act':
                nc.scalar.activation(h1, ph, RELU)
            else:
                nc.vector.tensor_scalar_max(h1, ph, 0.0)
            return h1

        def phi2_pair(h1, p, st):
            """phi2 MMs + fused relu/pool evacuation for pair p."""
            for mh in range(2):
                ph2 = psp.tile([128, bt], F32, tag="ph2", name="ph2", bufs=4)
                for k in range(2):
                    nc.tensor.matmul(
                        ph2,
                        w2sb[:, (2 * k + mh) * 128:(2 * k + mh + 1) * 128],
                        h1[:, k * bt:(k + 1) * bt],
                        start=(k == 0), stop=(k == 1),
                    )
                route = PH2_ROUTE[mh][p]
                acc_mh = st["acc"][:, mh * bt:(mh + 1) * bt]
                if route == 'stt':
                    if st["init"][mh]:
                        nc.vector.tensor_scalar(
                            acc_mh, ph2, nb2sb[:, mh:mh + 1], 0.0,
                            op0=AMAX, op1=ABYP)
                    else:
                        nc.vector.scalar_tensor_tensor(
                            acc_mh, ph2, nb2sb[:, mh:mh + 1], acc_mh,
                            op0=AMAX, op1=AADD)
                else:  # 'act' route: true relu(z + b2)
                    if st["init"][mh]:
                        nc.scalar.activation(acc_mh, ph2, RELU,
                                             bias=b2psb[:, mh:mh + 1])
                    else:
                        r = sbp.tile([128, bt], BF16, tag="rtmp", name="r",
                                     bufs=3)
                        nc.scalar.activation(r, ph2, RELU,
                                             bias=b2psb[:, mh:mh + 1])
                        e = st["add_eng"][mh].pop(0)
                        eng(e).tensor_add(acc_mh, acc_mh, r)
                st["init"][mh] = False

        def start_tile_state(t):
            acc = sbp.tile([128, 2 * bt], BF16, tag="acc", name="acc", bufs=2)
            return {
                "t": t, "acc": acc, "init": [True, True],
                "add_eng": [list(ADD_ENG[0]), list(ADD_ENG[1])],
            }

        def finisher(st):
            """rho + heads + clip + store, split into 3 stages that run
            interleaved with the next tile's duos."""
            t = st["t"]
            s0 = t * bt
            acc = st["acc"]
            fstate = {}

            def stage_a():  # rho matmuls + rho evac
                prs = []
                for m in range(2):
                    pr = psp.tile([128, bt], F32, tag="ph2", name="pr", bufs=4)
                    for k in range(2):
                        nc.tensor.matmul(
                            pr,
                            wrsb[:, (2 * k + m) * 128:(2 * k + m + 1) * 128],
                            acc[:, k * bt:(k + 1) * bt],
                            start=(k == 0), stop=(k == 1),
                        )
                    prs.append(pr)
                xs = sbp.tile([128, 2 * bt], BF16, tag="xs", name="xs", bufs=2)
                for m in range(2):
                    if RHO_ENG[m] == 'act':
                        nc.scalar.activation(
                            xs[:, m * bt:(m + 1) * bt], prs[m],
                            RELU, bias=brpsb[:, m:m + 1])
                    else:
                        nc.vector.tensor_scalar(
                            xs[:, m * bt:(m + 1) * bt], prs[m],
                            brpsb[:, m:m + 1], 0.0, op0=AADD, op1=AMAX)
                fstate["xs"] = xs

            def stage_b():  # head matmuls
                xs = fstate["xs"]
                py = psp.tile([8, bt], F32, tag="ph2", name="py", bufs=4)
                for k in range(2):
                    nc.tensor.matmul(
                        py, whsb[:, k * 8:(k + 1) * 8],
                        xs[:, k * bt:(k + 1) * bt],
                        start=(k == 0), stop=False,
                    )
                nc.tensor.matmul(py, bhsb, ones_sb, start=False, stop=True)
                fstate["py"] = py

            def stage_c():  # clip + store
                py = fstate["py"]
                ysb = sbp.tile([8, bt], F32, tag="ysb", name="ysb")
                nc.vector.tensor_scalar(
                    ysb, py, clipsb[:, 0:1], clipsb[:, 1:2],
                    op0=AMIN, op1=AMAX,
                )
                nc.sync.dma_start(out=y_d[:, s0:s0 + bt], in_=ysb)

            return [stage_a, stage_b, stage_c]

        # ---------- main pipeline ----------
        prev = None            # ((h1A, h1B), state, duo_idx) of previous duo
        pending_fin = None     # finisher stages of previous tile
        xts = dma_xts(0)
        xts_next = None
        for t in range(nt):
            st = start_tile_state(t)
            if t + 1 < nt:
                xts_next = dma_xts(t + 1)
            for d in range(3):
                phA, phB = phi1_mms(xts[d])
                h1A = h1_evac(phA, 2 * d)
                h1B = h1_evac(phB, 2 * d + 1)
                if prev is not None:
                    (pa, pb), pst, pd = prev
                    phi2_pair(pa, 2 * pd, pst)
                    phi2_pair(pb, 2 * pd + 1, pst)
                if pending_fin:
                    pending_fin[d]()
                prev = ((h1A, h1B), st, d)
            pending_fin = finisher(st)
            if t + 1 < nt:
                xts = xts_next
        # flush: last duo's phi2, then the final finisher
        (pa, pb), pst, pd = prev
        phi2_pair(pa, 2 * pd, pst)
        phi2_pair(pb, 2 * pd + 1, pst)
        for s in pending_fin:
            s()

    return nc


def _get_nc(bc, bt):
    key = (bc, bt)
    if key not in _CACHE:
        nc = _build_bass(bc, bt)
        nc.finalize()
        _CACHE[key] = nc
    return _CACHE[key]


def kernel(obs, ag, g, phi_w1, phi_b1, phi_w2, phi_b2,
           rho_w1, rho_b1, mean_w, mean_b, logstd_w, logstd_b):
    obs = np.asarray(obs, np.float32)
    ag = np.asarray(ag, np.float32)
    g = np.asarray(g, np.float32)
    B = obs.shape[0]
    assert B == B_FULL, f"kernel hardcoded for B={B_FULL}, got {B}"

    packed = _pack_weights(phi_w1, phi_b1, phi_w2, phi_b2, rho_w1, rho_b1,
                           mean_w, mean_b, logstd_w, logstd_b)
    xt3 = _pack_xt3(obs, ag, g)

    nc = _get_nc(BC, BT)
    in_maps = []
    for c in range(N_CORES):
        m = dict(packed)
        m["xt3"] = np.ascontiguousarray(xt3[:, :, c * BC:(c + 1) * BC])
        in_maps.append(m)

    import os
    trace = bool(os.environ.get("KERNEL_TRACE"))
    res = run_bass_kernel_spmd(nc, in_maps, core_ids=list(range(N_CORES)),
                               trace=trace)
    global _last_results
    _last_results = res

    y = np.concatenate([res.results[c]["y"] for c in range(N_CORES)], axis=1)
    out = np.ascontiguousarray(y.T)  # [B, 8]
    mean = out[:, 0:4].copy()
    logstd = out[:, 4:8].copy()
    return mean, logstd


_last_results = None


# revision 5
# speedup vs baseline: 1.0087x; 1.0087x over previous
"""Trainium2 Bass kernel for nn_ContinuousActor (GNN message passing actor MLP).

Strategy (pure data parallel over 8 cores, batch dim sharded, feature-major):
  - Host repacks per-pair inputs: pack(i,j) = [body(10); ones(1); A_i(24);
    A_j(24)] (K=59) where A_o = [ag_o(3); g_o(3); onehot_o(3); obj_o(15)].
    All 6 pairs share ONE stationary phi1 weight block [59, 256] (bias via
    the ones row, one-hots as data): the pair permutation becomes pure host
    data movement and phi1 needs no per-pair weights.
  - Two pairs ("duo") sit at SBUF partitions 0..58 / 64..122 and run as
    CONCURRENT matmuls via tile_position (0,0)/(64,0): phi1 costs ~half.
  - Batch tile 1024 (matmuls stay N=512 per PSUM bank): all PSUM-evacuation
    ops run at free-dim 1024 to amortize the fixed per-op engine overhead.
  - phi2 relu+sum-pool fused into DVE scalar_tensor_tensor ops:
      acc = (ph2 max -b2) add acc      (= relu(ph2+b2) - b2, accumulated)
    The constant -n_shift*b2 is folded into the rho bias host-side. The
    other half of the pairs use ACT relu (+b2 bias) with GPSIMD adds.
  - Head bias + clip run on host (device clips against bias-shifted bounds);
    saves the bias matmul and keeps the device output path to one DVE op.
  - ~72 junk warm-up matmuls at program start (overlapping the input DMA
    preamble) push the PE HAM clock gate to 8/8 before real work arrives.
"""

import numpy as np
import ml_dtypes
from contextlib import ExitStack

import concourse.bass as bass
import concourse.mybir as mybir
import concourse.tile as tile
from concourse import bacc
from concourse.bass_utils import run_bass_kernel_spmd

F32 = mybir.dt.float32
BF16 = mybir.dt.bfloat16
RELU = mybir.ActivationFunctionType.Relu
NPBF16 = ml_dtypes.bfloat16

B_FULL = 65536
N_CORES = 8
BC = B_FULL // N_CORES  # 8192 batch rows per core
BT = 1024               # batch tile (2 x 512-wide matmul free dim)
KP = 59                 # packed per-pair feature rows
PERMS = [(0, 1), (0, 2), (1, 0), (1, 2), (2, 0), (2, 1)]
LOG_SIG_MIN, LOG_SIG_MAX = -20.0, 2.0
N_WARMUP_MM = 72

# --- engine routing (baked into build AND the rho bias correction) ---
# phi2 evacuation per (mh, pair): 'stt' = DVE fused max/add (shifted by -b2,
# corrected in rho bias), 'act' = ACT relu+bias (true value; non-initial
# pairs need a GPSIMD add).
PH2_ROUTE = [['stt'] * 6, ['act'] * 6]
# h1 evacuation engines per (duo, half): each duo-mh evacuates pair A and
# pair B as two parallel FD-1024 ops on opposite engines.
H1_ENG = [('act', 'dve'), ('dve', 'act')] * 3  # indexed by duo, then (A,B)

_CACHE = {}


def _pack_256(w):
    """[256, 256] -> [128, 512] with col block (2k+m) = w[k*128:, m*128:]."""
    out = np.empty((128, 512), dtype=np.float32)
    for k in range(2):
        for m in range(2):
            out[:, (2 * k + m) * 128:(2 * k + m + 1) * 128] = \
                w[k * 128:(k + 1) * 128, m * 128:(m + 1) * 128]
    return out


def _pack_weights(phi_w1, phi_b1, phi_w2, phi_b2, rho_w1, rho_b1,
                  mean_w, mean_b, logstd_w, logstd_b):
    f = np.float32
    W1 = np.asarray(phi_w1, f)
    blk = np.concatenate([
        W1[12:22],                          # body
        np.asarray(phi_b1, f)[None, :],     # bias via ones row
        W1[0:3], W1[6:9], W1[22:25], W1[25:40],    # A_i: ag, g, onehot, feats
        W1[3:6], W1[9:12], W1[40:43], W1[43:58],   # A_j
    ], axis=0)                              # [59, 256]
    w1 = np.zeros((128, 256), dtype=f)
    w1[0:KP] = blk
    w1[64:64 + KP] = blk

    w2 = _pack_256(np.asarray(phi_w2, f))
    b2 = np.asarray(phi_b2, f)

    wr = _pack_256(np.asarray(rho_w1, f))
    # rho bias corrected for the 'stt'-shifted routes (acc is short of
    # n_shift*b2 on those feature halves).
    c = np.zeros(256, dtype=f)
    c[0:128] = sum(1 for r in PH2_ROUTE[0] if r == 'stt') * b2[0:128]
    c[128:256] = sum(1 for r in PH2_ROUTE[1] if r == 'stt') * b2[128:256]
    brv = np.asarray(rho_b1, f) + c @ np.asarray(rho_w1, f)

    wh_full = np.concatenate([np.asarray(mean_w, f), np.asarray(logstd_w, f)],
                             axis=1)                      # [256, 8]
    wh = np.concatenate([wh_full[0:128, :], wh_full[128:256, :]], axis=1)
    bh = np.concatenate([np.asarray(mean_b, f),
                         np.asarray(logstd_b, f)]).astype(f)  # [8]

    # bf16 const block: w1 | w2 | wr | wh  -> [128, 1296]
    cb = np.concatenate([w1, w2, wr, wh], axis=1).astype(NPBF16)
    # f32 const block: nb2(0:2) | b2p(2:4) | brp(4:6) | shifted clip(6:8)
    cf = np.zeros((128, 8), dtype=f)
    cf[:, 0] = -b2[0:128]
    cf[:, 1] = -b2[128:256]
    cf[:, 2] = b2[0:128]
    cf[:, 3] = b2[128:256]
    cf[:, 4] = brv[0:128]
    cf[:, 5] = brv[128:256]
    big = np.float32(3.0e38)
    hi = np.array([big] * 4 + [LOG_SIG_MAX] * 4, f) - bh
    lo = np.array([-big] * 4 + [LOG_SIG_MIN] * 4, f) - bh
    cf[0:8, 6] = hi
    cf[0:8, 7] = lo
    return dict(cb=cb, cf=cf, bh=bh)


def _pack_xt3(obs, ag, g):
    """[3, 128, B] bf16: duo d holds pair 2d at partitions 0..58 and pair
    2d+1 at partitions 64..122, each as [body;ones;A_i;A_j]."""
    B = obs.shape[0]
    xt3 = np.zeros((3, 128, B), dtype=NPBF16)
    bodyT = obs[:, 0:10].T.astype(NPBF16)
    agT = ag.T.astype(NPBF16)
    gT = g.T.astype(NPBF16)
    objT = [obs[:, 10 + 15 * o: 25 + 15 * o].T.astype(NPBF16) for o in range(3)]

    def fill_a(d, base, o):
        xt3[d, base:base + 3] = agT[3 * o:3 * o + 3]
        xt3[d, base + 3:base + 6] = gT[3 * o:3 * o + 3]
        xt3[d, base + 6 + o] = 1.0          # one-hot row
        xt3[d, base + 9:base + 24] = objT[o]

    for d in range(3):
        for half, p in ((0, 2 * d), (64, 2 * d + 1)):
            i, j = PERMS[p]
            xt3[d, half:half + 10] = bodyT
            xt3[d, half + 10] = 1.0
            fill_a(d, half + 11, i)
            fill_a(d, half + 35, j)
    return xt3


def _build_bass(bc, bt):
    nt = bc // bt
    nc = bacc.Bacc(trn_type="TRN2")

    xt3_d = nc.dram_tensor("xt3", [3, 128, bc], BF16, kind="ExternalInput")
    cb_d = nc.dram_tensor("cb", [128, 1296], BF16, kind="ExternalInput")
    cf_d = nc.dram_tensor("cf", [128, 8], F32, kind="ExternalInput")
    y_d = nc.dram_tensor("y", [8, bc], F32, kind="ExternalOutput")

    AMIN, AMAX, AADD = (mybir.AluOpType.min, mybir.AluOpType.max,
                        mybir.AluOpType.add)
    ABYP = mybir.AluOpType.bypass
    HB = bt // 2  # 512: matmul free dim / PSUM bank width

    with ExitStack() as ctx:
        tc = ctx.enter_context(tile.TileContext(nc))
        consts = ctx.enter_context(tc.tile_pool(name="consts", bufs=1))
        sbp = ctx.enter_context(tc.tile_pool(name="sbp", bufs=3))
        psp = ctx.enter_context(tc.tile_pool(name="psp", bufs=1, space="PSUM"))

        # --- warm-up: junk matmuls to lift the PE clock gate during DMA ---
        jw = consts.tile([128, 128], BF16, name="jw")
        nc.gpsimd.memset(jw, 0)
        wtile = psp.tile([64, 128], F32, tag="ph1", name="wtile", bufs=1,
                         padded_shape=[128, 2 * bt])
        for _ in range(N_WARMUP_MM):
            nc.tensor.matmul(wtile, jw[:, 0:64], jw, start=True, stop=True)

        # --- input DMAs first (first tile), then consts ---
        def dma_xts(t):
            s0 = t * bt
            xts = []
            for d in range(3):
                x = sbp.tile([128, bt], BF16, tag=f"xts{d}", name=f"xts{d}",
                             bufs=2)
                nc.sync.dma_start(out=x, in_=xt3_d[d, :, s0:s0 + bt])
                xts.append(x)
            return xts

        xts = dma_xts(0)
        cbsb = consts.tile([128, 1296], BF16, name="cbsb")
        nc.sync.dma_start(out=cbsb, in_=cb_d[:, :])
        cfsb = consts.tile([128, 8], F32, name="cfsb")
        nc.sync.dma_start(out=cfsb, in_=cf_d[:, :])

        w1sb = cbsb[:, 0:256]
        w2sb = cbsb[:, 256:768]
        wrsb = cbsb[:, 768:1280]
        whsb = cbsb[:, 1280:1296]
        nb2 = cfsb[:, 0:2]
        b2p = cfsb[:, 2:4]
        brp = cfsb[:, 4:6]
        clipsb = cfsb[:, 6:8]

        def eng(name):
            return {'act': nc.scalar, 'dve': nc.vector, 'gp': nc.gpsimd}[name]

        def phi1_half(xts_d, h1duo, d, mh):
            """One duo-mh: 4 MMs (two concurrent pairs x two batch halves)
            into a [128, 2*HB] psum tile, then 2 parallel FD-bt evacuations
            into h1duo[:, mh*2048 : (mh+1)*2048]."""
            ph = psp.tile([128, 2 * bt], F32, tag="ph1", name="ph", bufs=1)
            for bh in range(2):
                nc.tensor.matmul(
                    ph[:, 0 * bt + bh * HB:0 * bt + (bh + 1) * HB],
                    w1sb[0:KP, mh * 128:(mh + 1) * 128],
                    xts_d[0:KP, bh * HB:(bh + 1) * HB],
                    start=True, stop=True, tile_position=(0, 0),
                )
                nc.tensor.matmul(
                    ph[:, 1 * bt + bh * HB:1 * bt + (bh + 1) * HB],
                    w1sb[64:64 + KP, mh * 128:(mh + 1) * 128],
                    xts_d[64:64 + KP, bh * HB:(bh + 1) * HB],
                    start=True, stop=True, tile_position=(64, 0),
                )
            ea, eb = H1_ENG[d]
            for half, e in ((0, ea), (1, eb)):
                dst = h1duo[:, mh * 2 * bt + half * bt:
                            mh * 2 * bt + (half + 1) * bt]
                src = ph[:, half * bt:(half + 1) * bt]
                if e == 'act':
                    nc.scalar.activation(dst, src, RELU)
                else:
                    nc.vector.tensor_scalar_max(dst, src, 0.0)

        def phi2_pair(h1duo, pairoff, p, st):
            """phi2 MMs + fused relu/pool evacuation for one pair.
            h1duo layout: [128, 4096] = (k=0: A-b0 A-b1 B-b0 B-b1 | k=1: ...)
            """
            for mh in range(2):
                ph2 = psp.tile([128, bt], F32, tag="ph2", name="ph2", bufs=2)
                for bh in range(2):
                    for k in range(2):
                        nc.tensor.matmul(
                            ph2[:, bh * HB:(bh + 1) * HB],
                            w2sb[:, (2 * k + mh) * 128:(2 * k + mh + 1) * 128],
                            h1duo[:, k * 2 * bt + pairoff * bt + bh * HB:
                                  k * 2 * bt + pairoff * bt + (bh + 1) * HB],
                            start=(k == 0), stop=(k == 1),
                        )
                route = PH2_ROUTE[mh][p]
                acc_mh = st["acc"][:, mh * bt:(mh + 1) * bt]
                if route == 'stt':
                    if st["init"][mh]:
                        nc.vector.tensor_scalar(
                            acc_mh, ph2, nb2[:, mh:mh + 1], 0.0,
                            op0=AMAX, op1=ABYP)
                    else:
                        nc.vector.scalar_tensor_tensor(
                            acc_mh, ph2, nb2[:, mh:mh + 1], acc_mh,
                            op0=AMAX, op1=AADD)
                else:  # 'act' route: true relu(z + b2)
                    if st["init"][mh]:
                        nc.scalar.activation(acc_mh, ph2, RELU,
                                             bias=b2p[:, mh:mh + 1])
                    else:
                        r = sbp.tile([128, bt], BF16, tag="rtmp", name="r",
                                     bufs=3)
                        nc.scalar.activation(r, ph2, RELU,
                                             bias=b2p[:, mh:mh + 1])
                        nc.gpsimd.tensor_add(acc_mh, acc_mh, r)
                st["init"][mh] = False

        def start_tile_state(t):
            acc = sbp.tile([128, 2 * bt], BF16, tag="acc", name="acc", bufs=2)
            return {"t": t, "acc": acc, "init": [True, True]}

        def finisher(st):
            """rho + heads + clip + store, as 3 stages interleaved with the
            next tile's duos."""
            t = st["t"]
            s0 = t * bt
            acc = st["acc"]
            fstate = {}

            def stage_a():  # rho matmuls + rho evac
                prs = []
                for m in range(2):
                    pr = psp.tile([128, bt], F32, tag="ph2", name="pr", bufs=2)
                    for bh in range(2):
                        for k in range(2):
                            nc.tensor.matmul(
                                pr[:, bh * HB:(bh + 1) * HB],
                                wrsb[:, (2 * k + m) * 128:
                                     (2 * k + m + 1) * 128],
                                acc[:, k * bt + bh * HB:k * bt + (bh + 1) * HB],
                                start=(k == 0), stop=(k == 1),
                            )
                    prs.append(pr)
                xs = sbp.tile([128, 2 * bt], BF16, tag="xs", name="xs", bufs=2)
                for m in range(2):
                    nc.scalar.activation(
                        xs[:, m * bt:(m + 1) * bt], prs[m],
                        RELU, bias=brp[:, m:m + 1])
                fstate["xs"] = xs

            def stage_b():  # head matmuls
                xs = fstate["xs"]
                py = psp.tile([8, bt], F32, tag="ph2", name="py", bufs=2)
                for bh in range(2):
                    for k in range(2):
                        nc.tensor.matmul(
                            py[:, bh * HB:(bh + 1) * HB],
                            whsb[:, k * 8:(k + 1) * 8],
                            xs[:, k * bt + bh * HB:k * bt + (bh + 1) * HB],
                            start=(k == 0), stop=(k == 1),
                        )
                fstate["py"] = py

            def stage_c():  # shifted clip + store (bias added on host)
                py = fstate["py"]
                ysb = sbp.tile([8, bt], F32, tag="ysb", name="ysb")
                nc.vector.tensor_scalar(
                    ysb, py, clipsb[0:8, 0:1], clipsb[0:8, 1:2],
                    op0=AMIN, op1=AMAX,
                )
                nc.sync.dma_start(out=y_d[:, s0:s0 + bt], in_=ysb)

            return [stage_a, stage_b, stage_c]

        # ---------- main pipeline ----------
        # Per duo: mh0 MMs+evac | first part of prev duo's phi2 | mh1 MMs+evac
        # | rest of prev phi2 | finisher stage of tile t-1.
        prev = None            # (h1duo, state, duo_idx) of previous duo
        pending_fin = None
        xts_next = None
        for t in range(nt):
            st = start_tile_state(t)
            if t + 1 < nt:
                xts_next = dma_xts(t + 1)
            for d in range(3):
                h1duo = sbp.tile([128, 2 * 2 * bt], BF16, tag="h1",
                                 name="h1duo", bufs=2)
                phi1_half(xts[d], h1duo, d, 0)
                if prev is not None:
                    (ph1s, pst, pd) = prev
                    phi2_pair(ph1s, 0, 2 * pd, pst)      # prev pair A
                phi1_half(xts[d], h1duo, d, 1)
                if prev is not None:
                    phi2_pair(ph1s, 1, 2 * pd + 1, pst)  # prev pair B
                if pending_fin:
                    pending_fin[d]()
                prev = (h1duo, st, d)
            pending_fin = finisher(st)
            if t + 1 < nt:
                xts = xts_next
        # flush: last duo's phi2, then the final finisher
        (ph1s, pst, pd) = prev
        phi2_pair(ph1s, 0, 2 * pd, pst)
        phi2_pair(ph1s, 1, 2 * pd + 1, pst)
        for s in pending_fin:
            s()

    return nc


def _get_nc(bc, bt):
    key = (bc, bt)
    if key not in _CACHE:
        nc = _build_bass(bc, bt)
        nc.finalize()
        _CACHE[key] = nc
    return _CACHE[key]


def kernel(obs, ag, g, phi_w1, phi_b1, phi_w2, phi_b2,
           rho_w1, rho_b1, mean_w, mean_b, logstd_w, logstd_b):
    obs = np.asarray(obs, np.float32)
    ag = np.asarray(ag, np.float32)
    g = np.asarray(g, np.float32)
    B = obs.shape[0]
    assert B == B_FULL, f"kernel hardcoded for B={B_FULL}, got {B}"

    packed = _pack_weights(phi_w1, phi_b1, phi_w2, phi_b2, rho_w1, rho_b1,
                           mean_w, mean_b, logstd_w, logstd_b)
    bh = packed.pop("bh")
    xt3 = _pack_xt3(obs, ag, g)

    nc = _get_nc(BC, BT)
    in_maps = []
    for c in range(N_CORES):
        m = dict(packed)
        m["xt3"] = np.ascontiguousarray(xt3[:, :, c * BC:(c + 1) * BC])
        in_maps.append(m)

    import os
    trace = bool(os.environ.get("KERNEL_TRACE"))
    res = run_bass_kernel_spmd(nc, in_maps, core_ids=list(range(N_CORES)),
                               trace=trace)
    global _last_results
    _last_results = res

    y = np.concatenate([res.results[c]["y"] for c in range(N_CORES)], axis=1)
    out = np.ascontiguousarray(y.T) + bh[None, :]  # host-side head bias
    mean = out[:, 0:4].copy()
    logstd = out[:, 4:8].copy()
    return mean, logstd


_last_results = None


# revision 11
# speedup vs baseline: 1.2936x; 1.2823x over previous
"""Trainium2 Bass kernel for nn_ContinuousActor (GNN message passing actor MLP).

Strategy (pure data parallel over 8 cores, batch dim sharded, feature-major):
  - Host repacks per-pair inputs: pack(i,j) = [body(10); ones(1); A_i(24);
    A_j(24)] (K=59) where A_o = [ag_o(3); g_o(3); onehot_o(3); obj_o(15)].
    All 6 pairs share ONE stationary phi1 weight block [59, 256] (bias via
    the ones row, one-hots as data): the pair permutation becomes pure host
    data movement and phi1 needs no per-pair weights.
  - Two pairs ("duo") sit at SBUF partitions 0..58 / 64..122 and run as
    CONCURRENT matmuls via tile_position (0,0)/(64,0): phi1 costs ~half.
  - Batch tile 1024 (matmuls stay N=512 per PSUM bank): all PSUM-evacuation
    ops run at free-dim 1024 to amortize the fixed per-op engine overhead.
  - phi2 relu+sum-pool fused into DVE scalar_tensor_tensor ops:
      acc = (ph2 max -b2) add acc      (= relu(ph2+b2) - b2, accumulated)
    The constant -n_shift*b2 is folded into the rho bias host-side. The
    other half of the pairs use ACT relu (+b2 bias) with GPSIMD adds.
  - Head bias + clip run on host (device clips against bias-shifted bounds);
    saves the bias matmul and keeps the device output path to one DVE op.
  - ~72 junk warm-up matmuls at program start (overlapping the input DMA
    preamble) push the PE HAM clock gate to 8/8 before real work arrives.
"""

import numpy as np
import ml_dtypes
from contextlib import ExitStack

import concourse.bass as bass
import concourse.mybir as mybir
import concourse.tile as tile
from concourse import bacc
from concourse.bass_utils import run_bass_kernel_spmd

F32 = mybir.dt.float32
BF16 = mybir.dt.bfloat16
RELU = mybir.ActivationFunctionType.Relu
NPBF16 = ml_dtypes.bfloat16

B_FULL = 65536
N_CORES = 8
BC = B_FULL // N_CORES  # 8192 batch rows per core
BT = 1024               # batch tile (2 x 512-wide matmul free dim)
KP = 59                 # packed per-pair feature rows
PERMS = [(0, 1), (0, 2), (1, 0), (1, 2), (2, 0), (2, 1)]
LOG_SIG_MIN, LOG_SIG_MAX = -20.0, 2.0
N_WARMUP_MM = 72

# --- engine routing (baked into build AND the rho bias correction) ---
# phi2 evacuation per (mh, pair): 'stt' = DVE fused max/add (shifted by -b2,
# corrected in rho bias), 'act' = ACT relu+bias (true value; non-initial
# pairs need a GPSIMD add).
PH2_ROUTE = [['stt'] * 6, ['act'] * 6]
# h1 evacuation engines per (duo, half): each duo-mh evacuates pair A and
# pair B as two parallel FD-1024 ops on opposite engines.
H1_ENG = [('act', 'dve'), ('dve', 'act')] * 3  # indexed by duo, then (A,B)

_CACHE = {}


def _pack_256(w):
    """[256, 256] -> [128, 512] with col block (2k+m) = w[k*128:, m*128:]."""
    out = np.empty((128, 512), dtype=np.float32)
    for k in range(2):
        for m in range(2):
            out[:, (2 * k + m) * 128:(2 * k + m + 1) * 128] = \
                w[k * 128:(k + 1) * 128, m * 128:(m + 1) * 128]
    return out


def _pack_weights(phi_w1, phi_b1, phi_w2, phi_b2, rho_w1, rho_b1,
                  mean_w, mean_b, logstd_w, logstd_b):
    f = np.float32
    W1 = np.asarray(phi_w1, f)
    blk = np.concatenate([
        W1[12:22],                          # body
        np.asarray(phi_b1, f)[None, :],     # bias via ones row
        W1[0:3], W1[6:9], W1[22:25], W1[25:40],    # A_i: ag, g, onehot, feats
        W1[3:6], W1[9:12], W1[40:43], W1[43:58],   # A_j
    ], axis=0)                              # [59, 256]
    w1 = np.zeros((128, 256), dtype=f)
    w1[0:KP] = blk
    w1[64:64 + KP] = blk

    w2 = _pack_256(np.asarray(phi_w2, f))
    b2 = np.asarray(phi_b2, f)

    wr = _pack_256(np.asarray(rho_w1, f))
    # rho bias corrected for the 'stt'-shifted routes (acc is short of
    # n_shift*b2 on those feature halves).
    c = np.zeros(256, dtype=f)
    c[0:128] = sum(1 for r in PH2_ROUTE[0] if r == 'stt') * b2[0:128]
    c[128:256] = sum(1 for r in PH2_ROUTE[1] if r == 'stt') * b2[128:256]
    brv = np.asarray(rho_b1, f) + c @ np.asarray(rho_w1, f)

    wh_full = np.concatenate([np.asarray(mean_w, f), np.asarray(logstd_w, f)],
                             axis=1)                      # [256, 8]
    wh = np.concatenate([wh_full[0:128, :], wh_full[128:256, :]], axis=1)
    bh = np.concatenate([np.asarray(mean_b, f),
                         np.asarray(logstd_b, f)]).astype(f)  # [8]

    # bf16 const block: w1 | w2 | wr | wh  -> [128, 1296]
    cb = np.concatenate([w1, w2, wr, wh], axis=1).astype(NPBF16)
    # f32 const block: nb2(0:2) | b2p(2:4) | brp(4:6) | shifted clip(6:8)
    cf = np.zeros((128, 8), dtype=f)
    cf[:, 0] = -b2[0:128]
    cf[:, 1] = -b2[128:256]
    cf[:, 2] = b2[0:128]
    cf[:, 3] = b2[128:256]
    cf[:, 4] = brv[0:128]
    cf[:, 5] = brv[128:256]
    big = np.float32(3.0e38)
    hi = np.array([big] * 4 + [LOG_SIG_MAX] * 4, f) - bh
    lo = np.array([-big] * 4 + [LOG_SIG_MIN] * 4, f) - bh
    cf[0:8, 6] = hi
    cf[0:8, 7] = lo
    return dict(cb=cb, cf=cf, bh=bh)


def _pack_xt3(obs, ag, g):
    """[3, 128, B] bf16: duo d holds pair 2d at partitions 0..58 and pair
    2d+1 at partitions 64..122, each as [body;ones;A_i;A_j]."""
    B = obs.shape[0]
    xt3 = np.zeros((3, 128, B), dtype=NPBF16)
    bodyT = obs[:, 0:10].T.astype(NPBF16)
    agT = ag.T.astype(NPBF16)
    gT = g.T.astype(NPBF16)
    objT = [obs[:, 10 + 15 * o: 25 + 15 * o].T.astype(NPBF16) for o in range(3)]

    def fill_a(d, base, o):
        xt3[d, base:base + 3] = agT[3 * o:3 * o + 3]
        xt3[d, base + 3:base + 6] = gT[3 * o:3 * o + 3]
        xt3[d, base + 6 + o] = 1.0          # one-hot row
        xt3[d, base + 9:base + 24] = objT[o]

    for d in range(3):
        for half, p in ((0, 2 * d), (64, 2 * d + 1)):
            i, j = PERMS[p]
            xt3[d, half:half + 10] = bodyT
            xt3[d, half + 10] = 1.0
            fill_a(d, half + 11, i)
            fill_a(d, half + 35, j)
    return xt3


def _build_bass(bc, bt):
    nt = bc // bt
    nc = bacc.Bacc(trn_type="TRN2")

    xt3_d = nc.dram_tensor("xt3", [3, 128, bc], BF16, kind="ExternalInput")
    cb_d = nc.dram_tensor("cb", [128, 1296], BF16, kind="ExternalInput")
    cf_d = nc.dram_tensor("cf", [128, 8], F32, kind="ExternalInput")
    y_d = nc.dram_tensor("y", [8, bc], F32, kind="ExternalOutput")

    AMIN, AMAX, AADD = (mybir.AluOpType.min, mybir.AluOpType.max,
                        mybir.AluOpType.add)
    ABYP = mybir.AluOpType.bypass
    HB = bt // 2  # 512: matmul free dim / PSUM bank width

    with ExitStack() as ctx:
        tc = ctx.enter_context(tile.TileContext(nc))
        consts = ctx.enter_context(tc.tile_pool(name="consts", bufs=1))
        sbp = ctx.enter_context(tc.tile_pool(name="sbp", bufs=3))
        psp = ctx.enter_context(tc.tile_pool(name="psp", bufs=1, space="PSUM"))

        # --- warm-up: junk matmuls to lift the PE clock gate during DMA ---
        jw = consts.tile([128, 128], BF16, name="jw")
        nc.gpsimd.memset(jw, 0)
        wtile = psp.tile([64, 128], F32, tag="ps", name="wtile", bufs=4,
                         padded_shape=[128, bt])
        for _ in range(N_WARMUP_MM):
            nc.tensor.matmul(wtile, jw[:, 0:64], jw, start=True, stop=True)

        # --- input DMAs first (first tile), then consts ---
        def dma_xts(t):
            s0 = t * bt
            xts = []
            for d in range(3):
                x = sbp.tile([128, bt], BF16, tag=f"xts{d}", name=f"xts{d}",
                             bufs=2)
                nc.sync.dma_start(out=x, in_=xt3_d[d, :, s0:s0 + bt])
                xts.append(x)
            return xts

        xts = dma_xts(0)
        cbsb = consts.tile([128, 1296], BF16, name="cbsb")
        nc.sync.dma_start(out=cbsb, in_=cb_d[:, :])
        cfsb = consts.tile([128, 8], F32, name="cfsb")
        nc.sync.dma_start(out=cfsb, in_=cf_d[:, :])

        w1sb = cbsb[:, 0:256]
        w2sb = cbsb[:, 256:768]
        wrsb = cbsb[:, 768:1280]
        whsb = cbsb[:, 1280:1296]
        nb2 = cfsb[:, 0:2]
        b2p = cfsb[:, 2:4]
        brp = cfsb[:, 4:6]
        clipsb = cfsb[:, 6:8]

        def eng(name):
            return {'act': nc.scalar, 'dve': nc.vector, 'gp': nc.gpsimd}[name]

        def phi1_half(xts_d, h1duo, d, mh):
            """One duo-mh: two [128, bt] psum tiles (pair A strip 0, pair B
            strip 64, concurrent MMs), each evacuated as one FD-bt op into
            h1duo[:, mh*2048 + pair*1024 :]."""
            phs = [psp.tile([128, bt], F32, tag="ps", name="ph", bufs=4)
                   for _ in range(2)]
            for bh in range(2):
                nc.tensor.matmul(
                    phs[0][:, bh * HB:(bh + 1) * HB],
                    w1sb[0:KP, mh * 128:(mh + 1) * 128],
                    xts_d[0:KP, bh * HB:(bh + 1) * HB],
                    start=True, stop=True, tile_position=(0, 0),
                )
                nc.tensor.matmul(
                    phs[1][:, bh * HB:(bh + 1) * HB],
                    w1sb[64:64 + KP, mh * 128:(mh + 1) * 128],
                    xts_d[64:64 + KP, bh * HB:(bh + 1) * HB],
                    start=True, stop=True, tile_position=(64, 0),
                )
            ea, eb = H1_ENG[d] if mh == 0 else H1_ENG[d][::-1]
            for pair, e in ((0, ea), (1, eb)):
                dst = h1duo[:, mh * 2 * bt + pair * bt:
                            mh * 2 * bt + (pair + 1) * bt]
                src = phs[pair]
                if e == 'act':
                    nc.scalar.activation(dst, src, RELU)
                else:
                    nc.vector.tensor_scalar_max(dst, src, 0.0)

        def phi2_pair(h1duo, pairoff, p, st):
            """phi2 MMs + fused relu/pool evacuation for one pair.
            h1duo layout: [128, 4096] = (k=0: A-b0 A-b1 B-b0 B-b1 | k=1: ...)
            """
            for mh in range(2):
                ph2 = psp.tile([128, bt], F32, tag="ps", name="ph2", bufs=4)
                for bh in range(2):
                    for k in range(2):
                        nc.tensor.matmul(
                            ph2[:, bh * HB:(bh + 1) * HB],
                            w2sb[:, (2 * k + mh) * 128:(2 * k + mh + 1) * 128],
                            h1duo[:, k * 2 * bt + pairoff * bt + bh * HB:
                                  k * 2 * bt + pairoff * bt + (bh + 1) * HB],
                            start=(k == 0), stop=(k == 1),
                        )
                route = PH2_ROUTE[mh][p]
                acc_mh = st["acc"][:, mh * bt:(mh + 1) * bt]
                if route == 'stt':
                    if st["init"][mh]:
                        nc.vector.tensor_scalar(
                            acc_mh, ph2, nb2[:, mh:mh + 1], 0.0,
                            op0=AMAX, op1=ABYP)
                    else:
                        nc.vector.scalar_tensor_tensor(
                            acc_mh, ph2, nb2[:, mh:mh + 1], acc_mh,
                            op0=AMAX, op1=AADD)
                else:  # 'act' route: true relu(z + b2)
                    if st["init"][mh]:
                        nc.scalar.activation(acc_mh, ph2, RELU,
                                             bias=b2p[:, mh:mh + 1])
                    else:
                        r = sbp.tile([128, bt], BF16, tag="rtmp", name="r",
                                     bufs=4)
                        nc.scalar.activation(r, ph2, RELU,
                                             bias=b2p[:, mh:mh + 1])
                        # pair-tree on GPSIMD: shallower dependency chain
                        # than a serial acc += r for every pair
                        if st["pend"][mh] is None:
                            st["pend"][mh] = r
                        else:
                            s = sbp.tile([128, bt], BF16, tag="rtmp",
                                         name="s", bufs=4)
                            nc.gpsimd.tensor_add(s, st["pend"][mh], r)
                            nc.gpsimd.tensor_add(acc_mh, acc_mh, s)
                            st["pend"][mh] = None
                st["init"][mh] = False
            if p == 5:  # flush leftover tree terms
                for mh in range(2):
                    if st["pend"][mh] is not None:
                        acc_mh = st["acc"][:, mh * bt:(mh + 1) * bt]
                        nc.gpsimd.tensor_add(acc_mh, acc_mh, st["pend"][mh])
                        st["pend"][mh] = None

        def start_tile_state(t):
            acc = sbp.tile([128, 2 * bt], BF16, tag="acc", name="acc", bufs=2)
            return {"t": t, "acc": acc, "init": [True, True],
                    "pend": [None, None]}

        def finisher(st):
            """rho + heads + clip + store, as 3 stages interleaved with the
            next tile's duos."""
            t = st["t"]
            s0 = t * bt
            acc = st["acc"]
            fstate = {}

            def stage_a():  # rho matmuls + rho evac
                prs = []
                for m in range(2):
                    pr = psp.tile([128, bt], F32, tag="ps", name="pr", bufs=4)
                    for bh in range(2):
                        for k in range(2):
                            nc.tensor.matmul(
                                pr[:, bh * HB:(bh + 1) * HB],
                                wrsb[:, (2 * k + m) * 128:
                                     (2 * k + m + 1) * 128],
                                acc[:, k * bt + bh * HB:k * bt + (bh + 1) * HB],
                                start=(k == 0), stop=(k == 1),
                            )
                    prs.append(pr)
                xs = sbp.tile([128, 2 * bt], BF16, tag="xs", name="xs", bufs=2)
                for m in range(2):
                    nc.scalar.activation(
                        xs[:, m * bt:(m + 1) * bt], prs[m],
                        RELU, bias=brp[:, m:m + 1])
                fstate["xs"] = xs

            def stage_b():  # head matmuls
                xs = fstate["xs"]
                py = psp.tile([8, bt], F32, tag="ps", name="py", bufs=4)
                for bh in range(2):
                    for k in range(2):
                        nc.tensor.matmul(
                            py[:, bh * HB:(bh + 1) * HB],
                            whsb[:, k * 8:(k + 1) * 8],
                            xs[:, k * bt + bh * HB:k * bt + (bh + 1) * HB],
                            start=(k == 0), stop=(k == 1),
                        )
                fstate["py"] = py

            def stage_c():  # shifted clip + store (bias added on host)
                py = fstate["py"]
                ysb = sbp.tile([8, bt], F32, tag="ysb", name="ysb")
                nc.vector.tensor_scalar(
                    ysb, py, clipsb[0:8, 0:1], clipsb[0:8, 1:2],
                    op0=AMIN, op1=AMAX,
                )
                nc.sync.dma_start(out=y_d[:, s0:s0 + bt], in_=ysb)

            return [stage_a, stage_b, stage_c]

        # ---------- main pipeline ----------
        # Per duo: mh0 MMs+evac | first part of prev duo's phi2 | mh1 MMs+evac
        # | rest of prev phi2 | finisher stage of tile t-1.
        prev = None            # (h1duo, state, duo_idx) of previous duo
        pending_fin = None
        xts_next = None
        for t in range(nt):
            st = start_tile_state(t)
            if t + 1 < nt:
                xts_next = dma_xts(t + 1)
            for d in range(3):
                h1duo = sbp.tile([128, 2 * 2 * bt], BF16, tag="h1",
                                 name="h1duo", bufs=2)
                phi1_half(xts[d], h1duo, d, 0)
                if prev is not None:
                    (ph1s, pst, pd) = prev
                    phi2_pair(ph1s, 0, 2 * pd, pst)      # prev pair A
                phi1_half(xts[d], h1duo, d, 1)
                if prev is not None:
                    phi2_pair(ph1s, 1, 2 * pd + 1, pst)  # prev pair B
                if pending_fin:
                    if d == 1:
                        pending_fin[0]()
                    elif d == 2:
                        pending_fin[1]()
                        pending_fin[2]()
                prev = (h1duo, st, d)
            pending_fin = finisher(st)
            if t + 1 < nt:
                xts = xts_next
        # flush: last duo's phi2, then the final finisher
        (ph1s, pst, pd) = prev
        phi2_pair(ph1s, 0, 2 * pd, pst)
        phi2_pair(ph1s, 1, 2 * pd + 1, pst)
        for s in pending_fin:
            s()

    return nc


def _get_nc(bc, bt):
    key = (bc, bt)
    if key not in _CACHE:
        nc = _build_bass(bc, bt)
        nc.finalize()
        _CACHE[key] = nc
    return _CACHE[key]


def kernel(obs, ag, g, phi_w1, phi_b1, phi_w2, phi_b2,
           rho_w1, rho_b1, mean_w, mean_b, logstd_w, logstd_b):
    obs = np.asarray(obs, np.float32)
    ag = np.asarray(ag, np.float32)
    g = np.asarray(g, np.float32)
    B = obs.shape[0]
    assert B == B_FULL, f"kernel hardcoded for B={B_FULL}, got {B}"

    packed = _pack_weights(phi_w1, phi_b1, phi_w2, phi_b2, rho_w1, rho_b1,
                           mean_w, mean_b, logstd_w, logstd_b)
    bh = packed.pop("bh")
    xt3 = _pack_xt3(obs, ag, g)

    nc = _get_nc(BC, BT)
    in_maps = []
    for c in range(N_CORES):
        m = dict(packed)
        m["xt3"] = np.ascontiguousarray(xt3[:, :, c * BC:(c + 1) * BC])
        in_maps.append(m)

    import os
    trace = bool(os.environ.get("KERNEL_TRACE"))
    res = run_bass_kernel_spmd(nc, in_maps, core_ids=list(range(N_CORES)),
                               trace=trace)
    global _last_results
    _last_results = res

    y = np.concatenate([res.results[c]["y"] for c in range(N_CORES)], axis=1)
    out = np.ascontiguousarray(y.T) + bh[None, :]  # host-side head bias
    mean = out[:, 0:4].copy()
    logstd = out[:, 4:8].copy()
    return mean, logstd


_last_results = None


# revision 12
# speedup vs baseline: 1.3396x; 1.0356x over previous
"""Trainium2 Bass kernel for nn_ContinuousActor (GNN message passing actor MLP).

Strategy (pure data parallel over 8 cores, batch dim sharded, feature-major):
  - Host repacks per-pair inputs: pack(i,j) = [body(10); ones(1); A_i(24);
    A_j(24)] (K=59) where A_o = [ag_o(3); g_o(3); onehot_o(3); obj_o(15)].
    All 6 pairs share ONE stationary phi1 weight block [59, 256] (bias via
    the ones row, one-hots as data): the pair permutation becomes pure host
    data movement and phi1 needs no per-pair weights.
  - Two pairs ("duo") sit at SBUF partitions 0..58 / 64..122 and run as
    CONCURRENT matmuls via tile_position (0,0)/(64,0): phi1 costs ~half.
  - Batch tile 1024 (matmuls stay N=512 per PSUM bank): all PSUM-evacuation
    ops run at free-dim 1024 to amortize the fixed per-op engine overhead.
  - phi2 relu+sum-pool fused into DVE scalar_tensor_tensor ops:
      acc = (ph2 max -b2) add acc      (= relu(ph2+b2) - b2, accumulated)
    The constant -n_shift*b2 is folded into the rho bias host-side. The
    other half of the pairs use ACT relu (+b2 bias) with GPSIMD adds.
  - Head bias + clip run on host (device clips against bias-shifted bounds);
    saves the bias matmul and keeps the device output path to one DVE op.
  - ~72 junk warm-up matmuls at program start (overlapping the input DMA
    preamble) push the PE HAM clock gate to 8/8 before real work arrives.
"""

import numpy as np
import ml_dtypes
from contextlib import ExitStack

import concourse.bass as bass
import concourse.mybir as mybir
import concourse.tile as tile
from concourse import bacc
from concourse.bass_utils import run_bass_kernel_spmd

F32 = mybir.dt.float32
BF16 = mybir.dt.bfloat16
RELU = mybir.ActivationFunctionType.Relu
NPBF16 = ml_dtypes.bfloat16

B_FULL = 65536
N_CORES = 8
BC = B_FULL // N_CORES  # 8192 batch rows per core
BT = 1024               # batch tile (2 x 512-wide matmul free dim)
KP = 59                 # packed per-pair feature rows
PERMS = [(0, 1), (0, 2), (1, 0), (1, 2), (2, 0), (2, 1)]
LOG_SIG_MIN, LOG_SIG_MAX = -20.0, 2.0
N_WARMUP_MM = 44

# --- engine routing (baked into build AND the rho bias correction) ---
# phi2 evacuation per (mh, pair): 'stt' = DVE fused max/add (shifted by -b2,
# corrected in rho bias), 'act' = ACT relu+bias (true value; non-initial
# pairs need a GPSIMD add).
PH2_ROUTE = [['stt'] * 6, ['act'] * 6]
# h1 evacuation engines per (duo, half): each duo-mh evacuates pair A and
# pair B as two parallel FD-1024 ops on opposite engines.
H1_ENG = [('act', 'dve'), ('dve', 'act')] * 3  # indexed by duo, then (A,B)

_CACHE = {}


def _pack_256(w):
    """[256, 256] -> [128, 512] with col block (2k+m) = w[k*128:, m*128:]."""
    out = np.empty((128, 512), dtype=np.float32)
    for k in range(2):
        for m in range(2):
            out[:, (2 * k + m) * 128:(2 * k + m + 1) * 128] = \
                w[k * 128:(k + 1) * 128, m * 128:(m + 1) * 128]
    return out


def _pack_weights(phi_w1, phi_b1, phi_w2, phi_b2, rho_w1, rho_b1,
                  mean_w, mean_b, logstd_w, logstd_b):
    f = np.float32
    W1 = np.asarray(phi_w1, f)
    blk = np.concatenate([
        W1[12:22],                          # body
        np.asarray(phi_b1, f)[None, :],     # bias via ones row
        W1[0:3], W1[6:9], W1[22:25], W1[25:40],    # A_i: ag, g, onehot, feats
        W1[3:6], W1[9:12], W1[40:43], W1[43:58],   # A_j
    ], axis=0)                              # [59, 256]
    w1 = np.zeros((128, 256), dtype=f)
    w1[0:KP] = blk
    w1[64:64 + KP] = blk

    w2 = _pack_256(np.asarray(phi_w2, f))
    b2 = np.asarray(phi_b2, f)

    wr = _pack_256(np.asarray(rho_w1, f))
    # rho bias corrected for the 'stt'-shifted routes (acc is short of
    # n_shift*b2 on those feature halves).
    c = np.zeros(256, dtype=f)
    c[0:128] = sum(1 for r in PH2_ROUTE[0] if r == 'stt') * b2[0:128]
    c[128:256] = sum(1 for r in PH2_ROUTE[1] if r == 'stt') * b2[128:256]
    brv = np.asarray(rho_b1, f) + c @ np.asarray(rho_w1, f)

    wh_full = np.concatenate([np.asarray(mean_w, f), np.asarray(logstd_w, f)],
                             axis=1)                      # [256, 8]
    wh = np.concatenate([wh_full[0:128, :], wh_full[128:256, :]], axis=1)
    bh = np.concatenate([np.asarray(mean_b, f),
                         np.asarray(logstd_b, f)]).astype(f)  # [8]

    # bf16 const block: w1 | w2 | wr | wh  -> [128, 1296]
    cb = np.concatenate([w1, w2, wr, wh], axis=1).astype(NPBF16)
    # f32 const block: nb2(0:2) | b2p(2:4) | brp(4:6) | shifted clip(6:8)
    cf = np.zeros((128, 8), dtype=f)
    cf[:, 0] = -b2[0:128]
    cf[:, 1] = -b2[128:256]
    cf[:, 2] = b2[0:128]
    cf[:, 3] = b2[128:256]
    cf[:, 4] = brv[0:128]
    cf[:, 5] = brv[128:256]
    big = np.float32(3.0e38)
    hi = np.array([big] * 4 + [LOG_SIG_MAX] * 4, f) - bh
    lo = np.array([-big] * 4 + [LOG_SIG_MIN] * 4, f) - bh
    cf[0:8, 6] = hi
    cf[0:8, 7] = lo
    return dict(cb=cb, cf=cf, bh=bh)


def _pack_xt3(obs, ag, g):
    """[3, 128, B] bf16: duo d holds pair 2d at partitions 0..58 and pair
    2d+1 at partitions 64..122, each as [body;ones;A_i;A_j]."""
    B = obs.shape[0]
    xt3 = np.zeros((3, 128, B), dtype=NPBF16)
    bodyT = obs[:, 0:10].T.astype(NPBF16)
    agT = ag.T.astype(NPBF16)
    gT = g.T.astype(NPBF16)
    objT = [obs[:, 10 + 15 * o: 25 + 15 * o].T.astype(NPBF16) for o in range(3)]

    def fill_a(d, base, o):
        xt3[d, base:base + 3] = agT[3 * o:3 * o + 3]
        xt3[d, base + 3:base + 6] = gT[3 * o:3 * o + 3]
        xt3[d, base + 6 + o] = 1.0          # one-hot row
        xt3[d, base + 9:base + 24] = objT[o]

    for d in range(3):
        for half, p in ((0, 2 * d), (64, 2 * d + 1)):
            i, j = PERMS[p]
            xt3[d, half:half + 10] = bodyT
            xt3[d, half + 10] = 1.0
            fill_a(d, half + 11, i)
            fill_a(d, half + 35, j)
    return xt3


def _build_bass(bc, bt):
    nt = bc // bt
    nc = bacc.Bacc(trn_type="TRN2")

    xt3_d = nc.dram_tensor("xt3", [3, 128, bc], BF16, kind="ExternalInput")
    cb_d = nc.dram_tensor("cb", [128, 1296], BF16, kind="ExternalInput")
    cf_d = nc.dram_tensor("cf", [128, 8], F32, kind="ExternalInput")
    y_d = nc.dram_tensor("y", [8, bc], F32, kind="ExternalOutput")

    AMIN, AMAX, AADD = (mybir.AluOpType.min, mybir.AluOpType.max,
                        mybir.AluOpType.add)
    ABYP = mybir.AluOpType.bypass
    HB = bt // 2  # 512: matmul free dim / PSUM bank width

    with ExitStack() as ctx:
        tc = ctx.enter_context(tile.TileContext(nc))
        consts = ctx.enter_context(tc.tile_pool(name="consts", bufs=1))
        sbp = ctx.enter_context(tc.tile_pool(name="sbp", bufs=3))
        psp = ctx.enter_context(tc.tile_pool(name="psp", bufs=1, space="PSUM"))

        # --- warm-up: junk matmuls to lift the PE clock gate during DMA ---
        jw = consts.tile([128, 128], BF16, name="jw")
        nc.gpsimd.memset(jw, 0)
        wtile = psp.tile([64, 128], F32, tag="ps", name="wtile", bufs=4,
                         padded_shape=[128, bt])
        for _ in range(N_WARMUP_MM):
            nc.tensor.matmul(wtile, jw[:, 0:64], jw, start=True, stop=True)

        # --- input DMAs first (first tile), then consts ---
        def dma_xts(t):
            s0 = t * bt
            xts = []
            for d in range(3):
                x = sbp.tile([128, bt], BF16, tag=f"xts{d}", name=f"xts{d}",
                             bufs=2)
                nc.sync.dma_start(out=x, in_=xt3_d[d, :, s0:s0 + bt])
                xts.append(x)
            return xts

        xts = dma_xts(0)
        cbsb = consts.tile([128, 1296], BF16, name="cbsb")
        nc.sync.dma_start(out=cbsb, in_=cb_d[:, :])
        cfsb = consts.tile([128, 8], F32, name="cfsb")
        nc.sync.dma_start(out=cfsb, in_=cf_d[:, :])

        w1sb = cbsb[:, 0:256]
        w2sb = cbsb[:, 256:768]
        wrsb = cbsb[:, 768:1280]
        whsb = cbsb[:, 1280:1296]
        nb2 = cfsb[:, 0:2]
        b2p = cfsb[:, 2:4]
        brp = cfsb[:, 4:6]
        clipsb = cfsb[:, 6:8]

        def eng(name):
            return {'act': nc.scalar, 'dve': nc.vector, 'gp': nc.gpsimd}[name]

        def phi1_half(xts_d, h1duo, d, mh):
            """One duo-mh: two [128, bt] psum tiles (pair A strip 0, pair B
            strip 64, concurrent MMs), each evacuated as one FD-bt op into
            h1duo[:, mh*2048 + pair*1024 :]."""
            phs = [psp.tile([128, bt], F32, tag="ps", name="ph", bufs=4)
                   for _ in range(2)]
            for bh in range(2):
                nc.tensor.matmul(
                    phs[0][:, bh * HB:(bh + 1) * HB],
                    w1sb[0:KP, mh * 128:(mh + 1) * 128],
                    xts_d[0:KP, bh * HB:(bh + 1) * HB],
                    start=True, stop=True, tile_position=(0, 0),
                )
                nc.tensor.matmul(
                    phs[1][:, bh * HB:(bh + 1) * HB],
                    w1sb[64:64 + KP, mh * 128:(mh + 1) * 128],
                    xts_d[64:64 + KP, bh * HB:(bh + 1) * HB],
                    start=True, stop=True, tile_position=(64, 0),
                )
            ea, eb = H1_ENG[d] if mh == 0 else H1_ENG[d][::-1]
            for pair, e in ((0, ea), (1, eb)):
                dst = h1duo[:, mh * 2 * bt + pair * bt:
                            mh * 2 * bt + (pair + 1) * bt]
                src = phs[pair]
                if e == 'act':
                    nc.scalar.activation(dst, src, RELU)
                else:
                    nc.vector.tensor_scalar_max(dst, src, 0.0)

        def phi2_pair(h1duo, pairoff, p, st):
            """phi2 MMs + fused relu/pool evacuation for one pair.
            h1duo layout: [128, 4096] = (k=0: A-b0 A-b1 B-b0 B-b1 | k=1: ...)
            """
            for mh in range(2):
                ph2 = psp.tile([128, bt], F32, tag="ps", name="ph2", bufs=4)
                for bh in range(2):
                    for k in range(2):
                        nc.tensor.matmul(
                            ph2[:, bh * HB:(bh + 1) * HB],
                            w2sb[:, (2 * k + mh) * 128:(2 * k + mh + 1) * 128],
                            h1duo[:, k * 2 * bt + pairoff * bt + bh * HB:
                                  k * 2 * bt + pairoff * bt + (bh + 1) * HB],
                            start=(k == 0), stop=(k == 1),
                        )
                route = PH2_ROUTE[mh][p]
                acc_mh = st["acc"][:, mh * bt:(mh + 1) * bt]
                if route == 'stt':
                    if st["init"][mh]:
                        nc.vector.tensor_scalar(
                            acc_mh, ph2, nb2[:, mh:mh + 1], 0.0,
                            op0=AMAX, op1=ABYP)
                    else:
                        nc.vector.scalar_tensor_tensor(
                            acc_mh, ph2, nb2[:, mh:mh + 1], acc_mh,
                            op0=AMAX, op1=AADD)
                else:  # 'act' route: true relu(z + b2)
                    if st["init"][mh]:
                        nc.scalar.activation(acc_mh, ph2, RELU,
                                             bias=b2p[:, mh:mh + 1])
                    else:
                        r = sbp.tile([128, bt], BF16, tag="rtmp", name="r",
                                     bufs=4)
                        nc.scalar.activation(r, ph2, RELU,
                                             bias=b2p[:, mh:mh + 1])
                        # pair-tree on GPSIMD: shallower dependency chain
                        # than a serial acc += r for every pair. The last
                        # tile drains the pipeline, so its adds go on the
                        # (then idle, much faster) DVE instead.
                        adder = nc.vector if st["last"] else nc.gpsimd
                        if st["pend"][mh] is None:
                            st["pend"][mh] = r
                        else:
                            s = sbp.tile([128, bt], BF16, tag="rtmp",
                                         name="s", bufs=4)
                            adder.tensor_add(s, st["pend"][mh], r)
                            adder.tensor_add(acc_mh, acc_mh, s)
                            st["pend"][mh] = None
                st["init"][mh] = False
            if p == 5:  # flush leftover tree terms
                adder = nc.vector if st["last"] else nc.gpsimd
                for mh in range(2):
                    if st["pend"][mh] is not None:
                        acc_mh = st["acc"][:, mh * bt:(mh + 1) * bt]
                        adder.tensor_add(acc_mh, acc_mh, st["pend"][mh])
                        st["pend"][mh] = None

        def start_tile_state(t):
            acc = sbp.tile([128, 2 * bt], BF16, tag="acc", name="acc", bufs=2)
            return {"t": t, "acc": acc, "init": [True, True],
                    "pend": [None, None], "last": t == nt - 1}

        def finisher(st):
            """rho + heads + clip + store, as 3 stages interleaved with the
            next tile's duos."""
            t = st["t"]
            s0 = t * bt
            acc = st["acc"]
            fstate = {}

            def stage_a():  # rho matmuls + rho evac
                prs = []
                for m in range(2):
                    pr = psp.tile([128, bt], F32, tag="ps", name="pr", bufs=4)
                    for bh in range(2):
                        for k in range(2):
                            nc.tensor.matmul(
                                pr[:, bh * HB:(bh + 1) * HB],
                                wrsb[:, (2 * k + m) * 128:
                                     (2 * k + m + 1) * 128],
                                acc[:, k * bt + bh * HB:k * bt + (bh + 1) * HB],
                                start=(k == 0), stop=(k == 1),
                            )
                    prs.append(pr)
                xs = sbp.tile([128, 2 * bt], BF16, tag="xs", name="xs", bufs=2)
                for m in range(2):
                    nc.scalar.activation(
                        xs[:, m * bt:(m + 1) * bt], prs[m],
                        RELU, bias=brp[:, m:m + 1])
                fstate["xs"] = xs

            def stage_b():  # head matmuls
                xs = fstate["xs"]
                py = psp.tile([8, bt], F32, tag="ps", name="py", bufs=4)
                for bh in range(2):
                    for k in range(2):
                        nc.tensor.matmul(
                            py[:, bh * HB:(bh + 1) * HB],
                            whsb[:, k * 8:(k + 1) * 8],
                            xs[:, k * bt + bh * HB:k * bt + (bh + 1) * HB],
                            start=(k == 0), stop=(k == 1),
                        )
                fstate["py"] = py

            def stage_c():  # shifted clip + store (bias added on host)
                py = fstate["py"]
                ysb = sbp.tile([8, bt], F32, tag="ysb", name="ysb")
                nc.vector.tensor_scalar(
                    ysb, py, clipsb[0:8, 0:1], clipsb[0:8, 1:2],
                    op0=AMIN, op1=AMAX,
                )
                nc.sync.dma_start(out=y_d[:, s0:s0 + bt], in_=ysb)

            return [stage_a, stage_b, stage_c]

        # ---------- main pipeline ----------
        # Per duo: mh0 MMs+evac | first part of prev duo's phi2 | mh1 MMs+evac
        # | rest of prev phi2 | finisher stage of tile t-1.
        prev = None            # (h1duo, state, duo_idx) of previous duo
        pending_fin = None
        xts_next = None
        for t in range(nt):
            st = start_tile_state(t)
            if t + 1 < nt:
                xts_next = dma_xts(t + 1)
            for d in range(3):
                h1duo = sbp.tile([128, 2 * 2 * bt], BF16, tag="h1",
                                 name="h1duo", bufs=2)
                phi1_half(xts[d], h1duo, d, 0)
                if prev is not None:
                    (ph1s, pst, pd) = prev
                    phi2_pair(ph1s, 0, 2 * pd, pst)      # prev pair A
                phi1_half(xts[d], h1duo, d, 1)
                if prev is not None:
                    phi2_pair(ph1s, 1, 2 * pd + 1, pst)  # prev pair B
                if pending_fin:
                    if d == 1:
                        pending_fin[0]()
                    elif d == 2:
                        pending_fin[1]()
                        pending_fin[2]()
                prev = (h1duo, st, d)
            pending_fin = finisher(st)
            if t + 1 < nt:
                xts = xts_next
        # flush: last duo's phi2, then the final finisher
        (ph1s, pst, pd) = prev
        phi2_pair(ph1s, 0, 2 * pd, pst)
        phi2_pair(ph1s, 1, 2 * pd + 1, pst)
        for s in pending_fin:
            s()

    return nc


def _get_nc(bc, bt):
    key = (bc, bt)
    if key not in _CACHE:
        nc = _build_bass(bc, bt)
        nc.finalize()
        _CACHE[key] = nc
    return _CACHE[key]


def kernel(obs, ag, g, phi_w1, phi_b1, phi_w2, phi_b2,
           rho_w1, rho_b1, mean_w, mean_b, logstd_w, logstd_b):
    obs = np.asarray(obs, np.float32)
    ag = np.asarray(ag, np.float32)
    g = np.asarray(g, np.float32)
    B = obs.shape[0]
    assert B == B_FULL, f"kernel hardcoded for B={B_FULL}, got {B}"

    packed = _pack_weights(phi_w1, phi_b1, phi_w2, phi_b2, rho_w1, rho_b1,
                           mean_w, mean_b, logstd_w, logstd_b)
    bh = packed.pop("bh")
    xt3 = _pack_xt3(obs, ag, g)

    nc = _get_nc(BC, BT)
    in_maps = []
    for c in range(N_CORES):
        m = dict(packed)
        m["xt3"] = np.ascontiguousarray(xt3[:, :, c * BC:(c + 1) * BC])
        in_maps.append(m)

    import os
    trace = bool(os.environ.get("KERNEL_TRACE"))
    res = run_bass_kernel_spmd(nc, in_maps, core_ids=list(range(N_CORES)),
                               trace=trace)
    global _last_results
    _last_results = res

    y = np.concatenate([res.results[c]["y"] for c in range(N_CORES)], axis=1)
    out = np.ascontiguousarray(y.T) + bh[None, :]  # host-side head bias
    mean = out[:, 0:4].copy()
    logstd = out[:, 4:8].copy()
    return mean, logstd


_last_results = None
